# revision 15
# baseline (speedup 1.0000x reference)
"""Trainium2 Bass kernel for CachedMultiHeadedAttention (tensor-parallel over heads).

Sharding: 8 cores x 4 heads. Each core computes Q projection + attention for
its 4 heads, then a partial output projection against its 512 rows of Wo.
Host sums the 8 partial outputs (the "all-reduce" done at unshard time) and
adds bo.

Device-side layouts are chosen so NO on-chip transposes are needed:
  - x is passed pre-transposed (xT [D, S]) so contraction dims land on
    SBUF partitions for every matmul.
  - k_cache is passed pre-transposed per head (kT [DK, pos]).
  - The reference's softmax quirk (softmax over the QUERY axis) maps to
    scores^T tiles [l_part, s_free]: one fused ACT pass does exp + row-sum.
    The 1/sum normalization is folded into V rows (8x less data than the
    weight matrix).
Matmuls run as float32r (full PE rate; plain fp32 matmul is 1/4 rate).
The k_new/v_new projections (rank-1 work) run in bf16 — they only affect one
of the 4096 cache rows, so their rounding is negligible in the output.

DMAs are consolidated into few large transfers: each dma_start costs ~625ns
on the single HWDGE queue, which was the dominant bottleneck of the naive
version (557 DMAs = 348us of serialized HWDGE time).
"""

import math

import numpy as np
import ml_dtypes

import concourse.bass as bass
import concourse.mybir as mybir
import concourse.tile as tile
from concourse import bacc
from concourse.bass_utils import run_bass_kernel_spmd

F32 = mybir.dt.float32
F32R = mybir.dt.float32r
BF16 = mybir.dt.bfloat16
AF = mybir.ActivationFunctionType

H, D, DK, S = 32, 4096, 128, 1024
NCORES = 8
HP = H // NCORES          # heads per core
DC = D // 128             # contraction chunks for d_model


def build(pos: int):
    L = pos + 1
    LC = (L + 127) // 128          # number of 128-wide l tiles
    LG = (LC + 7) // 8             # l-tile groups of 8 (1024 l per group)
    INV = 1.0 / math.sqrt(DK)

    nc = bacc.Bacc("TRN2", target_bir_lowering=False, debug=False,
                   num_devices=NCORES)

    xT_d = nc.dram_tensor("xT", [D, S], F32R, kind="ExternalInput").ap()
    wq_d = nc.dram_tensor("wq", [HP, D, DK], F32R, kind="ExternalInput").ap()
    wkv_d = nc.dram_tensor("wkv", [D, 2 * HP * DK], BF16, kind="ExternalInput").ap()
    xl_d = nc.dram_tensor("xl", [128, DC], BF16, kind="ExternalInput").ap()
    bq_d = nc.dram_tensor("bq", [HP, DK, 1], F32, kind="ExternalInput").ap()
    bkv_d = nc.dram_tensor("bkv", [1, 2 * HP * DK], F32, kind="ExternalInput").ap()
    kT_d = nc.dram_tensor("kT", [HP, DK, pos], F32R, kind="ExternalInput").ap()
    v_d = nc.dram_tensor("v", [HP, pos, DK], F32R, kind="ExternalInput").ap()
    wo_d = nc.dram_tensor("wo", [HP * DK, D], F32R, kind="ExternalInput").ap()
    out_d = nc.dram_tensor("out", [S, D], F32, kind="ExternalOutput").ap()

    with tile.TileContext(nc) as tc:
        # Pools are released LIFO; ctxT survives into the output projection,
        # so it sits at the bottom of the SBUF pool stack.
        ctxT_pool = tc.alloc_tile_pool(name="ctxT", bufs=1)
        xT_pool = tc.alloc_tile_pool(name="xT", bufs=1)
        qT_pool = tc.alloc_tile_pool(name="qT", bufs=2)
        small = tc.alloc_tile_pool(name="smallp", bufs=1)
        wq_pool = tc.alloc_tile_pool(name="wqp", bufs=3)
        wkv_pool = tc.alloc_tile_pool(name="wkvp", bufs=2)
        kt_pool = tc.alloc_tile_pool(name="ktp", bufs=2)
        v_pool = tc.alloc_tile_pool(name="vp", bufs=2)
        wt_pool = tc.alloc_tile_pool(name="wtp", bufs=3)
        vs_pool = tc.alloc_tile_pool(name="vsp", bufs=4)
        ss_pool = tc.alloc_tile_pool(name="ssp", bufs=8)

        psq = tc.alloc_tile_pool(name="psq", bufs=1, space="PSUM")
        kvn_pool = tc.alloc_tile_pool(name="kvn", bufs=1, space="PSUM")
        pss = psc = None   # allocated after kvn_pool is released

        # resident xT tiles: 8 big tiles of 4 chunks each (4KB descriptors)
        xbig = []
        for gx in range(DC // 4):
            xt = xT_pool.tile([128, 4 * S], F32R, name=f"xt{gx}", tag=f"xt{gx}")
            nc.sync.dma_start(
                xt[:], xT_d[gx * 512:(gx + 1) * 512, :].rearrange(
                    "(i p) s -> p i s", p=128))
            xbig.append(xt)

        def xsl(c, lo, sz):
            return xbig[c // 4][:, (c % 4) * S + lo:(c % 4) * S + lo + sz]

        ctxTs = [ctxT_pool.tile([128, S], F32R, name=f"cT{h}", tag=f"cT{h}")
                 for h in range(HP)]

        # biased k_new|v_new rows for all heads ([1, 2*HP*DK])
        kvrow = small.tile([1, 2 * HP * DK], F32R, name="kvrow", tag="kvrow")
        bkv_t = small.tile([1, 2 * HP * DK], F32, name="bkvt", tag="bkvt")
        nc.sync.dma_start(bkv_t[:], bkv_d[:])
        # x_last chunks in bf16: column c = x[-1, c*128:(c+1)*128]
        xl_t = small.tile([128, DC], BF16, name="xlt", tag="xlt")
        nc.sync.dma_start(xl_t[:], xl_d[:])

        # k_new/v_new psum accumulators (rows, all heads)
        kn_t = kvn_pool.tile([1, HP * DK], F32, name="knr", tag="kn", bufs=1)
        vn_t = kvn_pool.tile([1, HP * DK], F32, name="vnr", tag="vn", bufs=1)

        def emit_q_dma(h):
            """Issue the 8 weight-group DMAs for head h's Q projection."""
            wqts = []
            for gw in range(DC // 4):
                wqt = wq_pool.tile([128, 4 * DK], F32R,
                                   name=f"wq{h}_{gw}", tag="wq")
                nc.sync.dma_start(
                    wqt[:], wq_d[h, gw * 512:(gw + 1) * 512, :].rearrange(
                        "(i p) k -> p i k", p=128))
                wqts.append(wqt)
            return wqts

        def emit_q_mm(psq_t, wqts, c):
            lhs = wqts[c // 4][:, (c % 4) * DK:(c % 4 + 1) * DK]
            st, sp = (c == 0), (c == DC - 1)
            nc.tensor.matmul(psq_t[:, 0:512], lhs, xsl(c, 0, 512),
                             start=st, stop=sp)
            nc.tensor.matmul(psq_t[:, 512:1024], lhs, xsl(c, 512, 512),
                             start=st, stop=sp)

        def emit_qt(h, psq_t):
            bq_t = ss_pool.tile([128, 1], F32, name=f"bq{h}", tag="bq", bufs=2)
            nc.sync.dma_start(bq_t[:], bq_d[h])
            qT_t = qT_pool.tile([128, S], F32R, name=f"qT{h}", tag="qT")
            nc.vector.tensor_scalar_add(qT_t[:], psq_t[:], bq_t[:])
            return qT_t

        # ---------- head 0 Q projection (phase A, DMA-paced) ----------
        psq_t = psq.tile([128, S], F32, name="psq0", tag="psq")
        wqts = emit_q_dma(0)
        for c in range(DC):
            emit_q_mm(psq_t, wqts, c)
        qT_t = emit_qt(0, psq_t)

        pss = psc = None
        for h in range(HP):
            if h == 0:
                # rank-1 k_new/v_new projections (bf16) — emitted before the
                # S loop so the kvrow write precedes all kvrow readers in
                # trace order (Tile tracks deps in emission order).
                for c in range(DC):
                    wkvt = wkv_pool.tile([128, 2 * HP * DK], BF16,
                                         name=f"wkv{c}", tag="wkv")
                    nc.sync.dma_start(wkvt[:], wkv_d[c * 128:(c + 1) * 128, :])
                    st, sp = (c == 0), (c == DC - 1)
                    nc.tensor.matmul(kn_t[0:1, :], xl_t[:, c:c + 1],
                                     wkvt[:, 0:HP * DK], start=st, stop=sp)
                    nc.tensor.matmul(vn_t[0:1, :], xl_t[:, c:c + 1],
                                     wkvt[:, HP * DK:], start=st, stop=sp)
                nc.vector.tensor_add(kvrow[0:1, 0:HP * DK], kn_t[:],
                                     bkv_t[0:1, 0:HP * DK])
                nc.vector.tensor_add(kvrow[0:1, HP * DK:], vn_t[:],
                                     bkv_t[0:1, HP * DK:])
                kvn_pool.release()
                pss = tc.alloc_tile_pool(name="pss", bufs=2, space="PSUM")
                psc = tc.alloc_tile_pool(name="psc", bufs=1, space="PSUM")

            # ---------- attention head h, with head h+1's Q interleaved ----
            if h + 1 < HP:
                psq_t = psq.tile([128, S], F32, name=f"psq{h+1}", tag="psq")
                wqts = emit_q_dma(h + 1)
            psc_t = psc.tile([128, S], F32, name=f"psc{h}", tag="psc")
            kt8 = v8 = None
            pend = None              # lag-1 ctx: (lt, wt, vst)
            for lt in range(LC):
                g, j = lt // 8, lt % 8
                if j == 0:
                    # load l-group g: keys (16KB bursts) and values (512B)
                    g0 = g * 1024
                    gl = min(1024, L - g0)            # valid l in group
                    gc = max(0, min(1024, pos - g0))  # from cache
                    kt8 = kt_pool.tile([128, 1024], F32R,
                                       name=f"kt{h}_{g}", tag="kt")
                    if gc > 0:
                        nc.sync.dma_start(kt8[:, 0:gc], kT_d[h, :, g0:g0 + gc])
                    if gl > gc:   # new entry column
                        nc.sync.dma_start(kt8[:, gc:gc + 1],
                                          kvrow[0:1, h * DK:(h + 1) * DK])
                    if gl < 1024:
                        nc.vector.memset(kt8[:, gl:1024], 0.0)
                    v8 = v_pool.tile([128, 1024], F32R,
                                     name=f"v{h}_{g}", tag="v")
                    fc = gc // 128                    # full cache chunks
                    if fc > 0:
                        nc.sync.dma_start(
                            v8[:, 0:fc * 128],
                            v_d[h, g0:g0 + fc * 128, :].rearrange(
                                "(i p) k -> p i k", p=128))
                    rem = gc - fc * 128               # partial cache chunk
                    if rem > 0:
                        nc.sync.dma_start(
                            v8[0:rem, fc * 128:(fc + 1) * 128],
                            v_d[h, g0 + fc * 128:g0 + gc, :])
                    if gl > gc:   # new entry row
                        nc.sync.dma_start(
                            v8[gc % 128:gc % 128 + 1,
                               (gc // 128) * 128:(gc // 128 + 1) * 128],
                            kvrow[0:1, HP * DK + h * DK:HP * DK + (h + 1) * DK])
                    if gl < 1024:
                        for cc in range(gl // 128, 8):
                            lo = max(gl - cc * 128, 0)
                            if lo < 128:
                                nc.vector.memset(v8[lo:128, cc * 128:(cc + 1) * 128], 0.0)

                ps = pss.tile([128, 1024], F32, name=f"ps_{h}_{lt}", tag="pss")
                ksl = kt8[:, j * 128:(j + 1) * 128]
                nc.tensor.matmul(ps[:, 0:512], ksl, qT_t[:, 0:512])
                nc.tensor.matmul(ps[:, 512:1024], ksl, qT_t[:, 512:1024])

                # head h+1's Q projection rides in the ACT-bound S loop
                if h + 1 < HP and lt < DC:
                    emit_q_mm(psq_t, wqts, lt)

                wt = wt_pool.tile([128, 1024], F32R, name=f"wt_{h}_{lt}", tag="wt")
                ssum = ss_pool.tile([128, 1], F32, name=f"ss_{h}_{lt}", tag="ssum")
                nc.scalar.activation(wt[:], ps[:], AF.Exp, scale=INV, accum_out=ssum[:])
                rec = ss_pool.tile([128, 1], F32, name=f"rc_{h}_{lt}", tag="rec")
                nc.vector.reciprocal(rec[:], ssum[:])
                vst = vs_pool.tile([128, DK], F32R, name=f"vs{h}_{lt}", tag="vs")
                nc.vector.tensor_scalar_mul(vst[:], v8[:, j * 128:(j + 1) * 128], rec[:])

                if pend is not None:
                    plt, pwt, pvst = pend
                    nc.tensor.matmul(psc_t[:, 0:512], pvst[:], pwt[:, 0:512],
                                     start=(plt == 0), stop=False)
                    nc.tensor.matmul(psc_t[:, 512:1024], pvst[:], pwt[:, 512:1024],
                                     start=(plt == 0), stop=False)
                pend = (lt, wt, vst)
            plt, pwt, pvst = pend
            nc.tensor.matmul(psc_t[:, 0:512], pvst[:], pwt[:, 0:512],
                             start=(plt == 0), stop=True)
            nc.tensor.matmul(psc_t[:, 512:1024], pvst[:], pwt[:, 512:1024],
                             start=(plt == 0), stop=True)
            nc.vector.tensor_copy(ctxTs[h][:], psc_t[:])
            if h + 1 < HP:
                qT_t = emit_qt(h + 1, psq_t)

        # release attention-phase pools before the output projection (LIFO)
        for p in (psc, pss, psq,
                  ss_pool, vs_pool, wt_pool, v_pool, kt_pool,
                  wkv_pool, wq_pool, small, qT_pool, xT_pool):
            p.release()

        # ---------- output projection: out[s, m] partial ----------
        # Wo fully resident in the space freed by xT; one 16KB-burst output
        # DMA per s-tile.
        wo_pool = tc.alloc_tile_pool(name="wop", bufs=1)
        ob_pool = tc.alloc_tile_pool(name="obp", bufs=2)
        pso = tc.alloc_tile_pool(name="pso", bufs=4, space="PSUM")
        wos = []
        for c in range(HP):
            wot = wo_pool.tile([128, D], F32R, name=f"wo{c}", tag=f"wo{c}")
            nc.sync.dma_start(wot[:], wo_d[c * 128:(c + 1) * 128, :])
            wos.append(wot)
        for s_t in range(S // 128):
            ob = ob_pool.tile([128, D], F32, name=f"ob{s_t}", tag="ob")
            for mg in range(D // 512):
                pso_t = pso.tile([128, 512], F32, name=f"po{s_t}_{mg}", tag="pso")
                for c in range(HP):
                    nc.tensor.matmul(pso_t[:],
                                     ctxTs[c][:, s_t * 128:(s_t + 1) * 128],
                                     wos[c][:, mg * 512:(mg + 1) * 512],
                                     start=(c == 0), stop=(c == HP - 1))
                nc.vector.tensor_copy(ob[:, mg * 512:(mg + 1) * 512], pso_t[:])
            nc.sync.dma_start(out_d[s_t * 128:(s_t + 1) * 128, :], ob[:])
        for p in (pso, ob_pool, wo_pool, ctxT_pool):
            p.release()

    nc.compile()
    return nc


_CACHE = {}
LAST_EXEC_NS = None


def kernel(x, k_cache, v_cache, Wq, bq, Wk, bk, Wv, bv, Wo, bo, pos):
    global LAST_EXEC_NS
    pos = int(pos)

    def f32(a):
        return np.ascontiguousarray(np.asarray(a), dtype=np.float32)

    x = f32(x)
    k_cache, v_cache = f32(k_cache), f32(v_cache)
    Wq, Wk, Wv, Wo = f32(Wq), f32(Wk), f32(Wv), f32(Wo)
    bq, bk, bv, bo = f32(bq), f32(bk), f32(bv), f32(bo)

    xT = np.ascontiguousarray(x[0].T)              # [D, S]
    xl = np.ascontiguousarray(
        x[0, -1].reshape(DC, 128).T.astype(ml_dtypes.bfloat16))
    in_maps = []
    for i in range(NCORES):
        hs = slice(i * HP, (i + 1) * HP)
        in_maps.append({
            "xT": xT,
            "wq": np.ascontiguousarray(Wq[hs]),
            "wkv": np.ascontiguousarray(np.concatenate([
                Wk[hs].transpose(1, 0, 2).reshape(D, HP * DK),
                Wv[hs].transpose(1, 0, 2).reshape(D, HP * DK)],
                axis=1).astype(ml_dtypes.bfloat16)),
            "xl": xl,
            "bq": np.ascontiguousarray(bq[hs].reshape(HP, DK, 1)),
            "bkv": np.ascontiguousarray(np.concatenate(
                [bk[hs].reshape(-1), bv[hs].reshape(-1)])[None, :]),
            "kT": np.ascontiguousarray(k_cache[hs, :pos, :].transpose(0, 2, 1)),
            "v": np.ascontiguousarray(v_cache[hs, :pos, :]),
            "wo": np.ascontiguousarray(Wo[i * HP * DK:(i + 1) * HP * DK]),
        })

    if pos not in _CACHE:
        _CACHE[pos] = build(pos)
    nc = _CACHE[pos]

    res = run_bass_kernel_spmd(nc, in_maps, core_ids=list(range(NCORES)))
    LAST_EXEC_NS = res.exec_time_ns

    acc = np.zeros((S, D), np.float64)
    for r in res.results:
        acc += r["out"]
    out = (acc + bo.astype(np.float64)).astype(np.float32)
    return out[None]


# revision 19
# speedup vs baseline: 1.0811x; 1.0811x over previous
"""Trainium2 Bass kernel for CachedMultiHeadedAttention (tensor-parallel over heads).

Sharding: 8 cores x 4 heads. Each core computes Q projection + attention for
its 4 heads, then a partial output projection against its 512 rows of Wo.
Host sums the 8 partial outputs (the "all-reduce" done at unshard time) and
adds bo.

Device-side layouts are chosen so NO on-chip transposes are needed:
  - x is passed pre-transposed (xT [D, S]) so contraction dims land on
    SBUF partitions for every matmul.
  - k_cache is passed pre-transposed per head (kT [DK, pos]).
  - The reference's softmax quirk (softmax over the QUERY axis) maps to
    scores^T tiles [l_part, s_free]: one fused ACT pass does exp + row-sum.
    The 1/sum normalization is folded into V rows (8x less data than the
    weight matrix).
Matmuls run as float32r (full PE rate; plain fp32 matmul is 1/4 rate).
The k_new/v_new projections (rank-1 work) run in bf16 — they only affect one
of the 4096 cache rows, so their rounding is negligible in the output.

DMAs are consolidated into few large transfers: each dma_start costs ~625ns
on the single HWDGE queue, which was the dominant bottleneck of the naive
version (557 DMAs = 348us of serialized HWDGE time).
"""

import math

import numpy as np
import ml_dtypes

import concourse.bass as bass
import concourse.mybir as mybir
import concourse.tile as tile
from concourse import bacc
from concourse.bass_utils import run_bass_kernel_spmd

F32 = mybir.dt.float32
F32R = mybir.dt.float32r
BF16 = mybir.dt.bfloat16
AF = mybir.ActivationFunctionType

H, D, DK, S = 32, 4096, 128, 1024
NCORES = 8
HP = H // NCORES          # heads per core
DC = D // 128             # contraction chunks for d_model


def build(pos: int):
    L = pos + 1
    LC = (L + 127) // 128          # number of 128-wide l tiles
    LG = (LC + 7) // 8             # l-tile groups of 8 (1024 l per group)
    INV = 1.0 / math.sqrt(DK)

    nc = bacc.Bacc("TRN2", target_bir_lowering=False, debug=False,
                   num_devices=NCORES)

    xT_d = nc.dram_tensor("xT", [D, S], F32R, kind="ExternalInput").ap()
    wq_d = nc.dram_tensor("wq", [HP, D, DK], F32R, kind="ExternalInput").ap()
    wkv_d = nc.dram_tensor("wkv", [D, 2 * HP * DK], BF16, kind="ExternalInput").ap()
    xl_d = nc.dram_tensor("xl", [128, DC], BF16, kind="ExternalInput").ap()
    bq_d = nc.dram_tensor("bq", [HP, DK, 1], F32, kind="ExternalInput").ap()
    bkv_d = nc.dram_tensor("bkv", [1, 2 * HP * DK], F32, kind="ExternalInput").ap()
    kT_d = nc.dram_tensor("kT", [HP, DK, pos], F32R, kind="ExternalInput").ap()
    v_d = nc.dram_tensor("v", [HP, pos, DK], F32R, kind="ExternalInput").ap()
    wo_d = nc.dram_tensor("wo", [HP * DK, D], F32R, kind="ExternalInput").ap()
    out_d = nc.dram_tensor("out", [S, D], F32, kind="ExternalOutput").ap()

    with tile.TileContext(nc) as tc:
        # Pools are released LIFO; ctxT survives into the output projection,
        # so it sits at the bottom of the SBUF pool stack.
        ctxT_pool = tc.alloc_tile_pool(name="ctxT", bufs=1)
        xT_pool = tc.alloc_tile_pool(name="xT", bufs=1)
        qT_pool = tc.alloc_tile_pool(name="qT", bufs=2)
        small = tc.alloc_tile_pool(name="smallp", bufs=1)
        wq_pool = tc.alloc_tile_pool(name="wqp", bufs=8)
        wkv_pool = tc.alloc_tile_pool(name="wkvp", bufs=3)
        kt_pool = tc.alloc_tile_pool(name="ktp", bufs=2)
        v_pool = tc.alloc_tile_pool(name="vp", bufs=2)
        wt_pool = tc.alloc_tile_pool(name="wtp", bufs=2)
        vs_pool = tc.alloc_tile_pool(name="vsp", bufs=4)
        ss_pool = tc.alloc_tile_pool(name="ssp", bufs=8)

        # PSUM budget (8 banks): psq 1 + kv 1 + pss 4 + psc 2.
        # Q projections and the kv_new projections run in TWO s-half /
        # k-v passes so their accumulators are single-bank.
        psq = tc.alloc_tile_pool(name="psq", bufs=1, space="PSUM")
        kv_pool = tc.alloc_tile_pool(name="kvp", bufs=1, space="PSUM")
        pss = tc.alloc_tile_pool(name="pss", bufs=2, space="PSUM")
        psc = tc.alloc_tile_pool(name="psc", bufs=1, space="PSUM")

        ctxTs = [ctxT_pool.tile([128, S], F32R, name=f"cT{h}", tag=f"cT{h}")
                 for h in range(HP)]

        # small constants first (tiny DMAs, ahead of the big streams)
        kvrow = small.tile([1, 2 * HP * DK], F32R, name="kvrow", tag="kvrow")
        bkv_t = small.tile([1, 2 * HP * DK], F32, name="bkvt", tag="bkvt")
        nc.sync.dma_start(bkv_t[:], bkv_d[:])
        xl_t = small.tile([128, DC], BF16, name="xlt", tag="xlt")
        nc.sync.dma_start(xl_t[:], xl_d[:])

        # resident xT tiles (8 big tiles of 4 chunks), interleaved with head
        # 0's Q weight groups so the first Q matmuls start after ~2.5MB, not
        # after the full 17MB of x.
        xbig = []
        wq0_groups = []
        for gx in range(DC // 4):
            wqt = wq_pool.tile([128, 4 * DK], F32R, name=f"wq0_{gx}", tag="wq")
            nc.sync.dma_start(
                wqt[:], wq_d[0, gx * 512:(gx + 1) * 512, :].rearrange(
                    "(i p) k -> p i k", p=128))
            wq0_groups.append(wqt)
            xt = xT_pool.tile([128, 4 * S], F32R, name=f"xt{gx}", tag=f"xt{gx}")
            nc.sync.dma_start(
                xt[:], xT_d[gx * 512:(gx + 1) * 512, :].rearrange(
                    "(i p) s -> p i s", p=128))
            xbig.append(xt)

        def xsl(c, lo, sz):
            return xbig[c // 4][:, (c % 4) * S + lo:(c % 4) * S + lo + sz]

        def emit_wq_dma(h, gw, tag="wq"):
            wqt = wq_pool.tile([128, 4 * DK], F32R,
                               name=f"wq{h}_{gw}", tag=tag)
            nc.sync.dma_start(
                wqt[:], wq_d[h, gw * 512:(gw + 1) * 512, :].rearrange(
                    "(i p) k -> p i k", p=128))
            return wqt

        def q_half_mm(psq_t, wqt, c, half):
            lhs = wqt[:, (c % 4) * DK:(c % 4 + 1) * DK]
            nc.tensor.matmul(psq_t[:], lhs, xsl(c, half * 512, 512),
                             start=(c == 0), stop=(c == DC - 1))

        def q_half_add(h, qT_t, psq_t, half, bq_t):
            nc.vector.tensor_scalar_add(qT_t[:, half * 512:(half + 1) * 512],
                                        psq_t[:], bq_t[:])

        def kv_mm(kv_t, c, which):
            # which: 0 = k_new, 1 = v_new
            wkvt = wkv_pool.tile([128, HP * DK], BF16,
                                 name=f"wkv{which}_{c}", tag="wkv")
            nc.sync.dma_start(
                wkvt[:], wkv_d[c * 128:(c + 1) * 128,
                               which * HP * DK:(which + 1) * HP * DK])
            nc.tensor.matmul(kv_t[0:1, :], xl_t[:, c:c + 1], wkvt[:],
                             start=(c == 0), stop=(c == DC - 1))

        def kv_add(kv_t, which):
            nc.vector.tensor_add(
                kvrow[0:1, which * HP * DK:(which + 1) * HP * DK], kv_t[:],
                bkv_t[0:1, which * HP * DK:(which + 1) * HP * DK])

        def load_group(h, g):
            """Cache-only loads of l-group g (the new-entry writes are
            emitted separately, after kvrow's writes in trace order)."""
            g0 = g * 1024
            gl = min(1024, L - g0)            # valid l in group
            gc = max(0, min(1024, pos - g0))  # of which from cache
            kt8 = kt_pool.tile([128, 1024], F32R, name=f"kt{h}_{g}", tag="kt")
            if gc > 0:
                nc.sync.dma_start(kt8[:, 0:gc], kT_d[h, :, g0:g0 + gc])
            if gl < 1024:
                nc.vector.memset(kt8[:, gl:1024], 0.0)
            v8 = v_pool.tile([128, 1024], F32R, name=f"v{h}_{g}", tag="v")
            fc = gc // 128
            if fc > 0:
                nc.sync.dma_start(
                    v8[:, 0:fc * 128],
                    v_d[h, g0:g0 + fc * 128, :].rearrange(
                        "(i p) k -> p i k", p=128))
            rem = gc - fc * 128
            if rem > 0:
                nc.sync.dma_start(v8[0:rem, fc * 128:(fc + 1) * 128],
                                  v_d[h, g0 + fc * 128:g0 + gc, :])
            if gl < 1024:
                for cc in range(gl // 128, 8):
                    lo = max(gl - cc * 128, 0)
                    if lo < 128:
                        nc.vector.memset(v8[lo:128, cc * 128:(cc + 1) * 128], 0.0)
            return kt8, v8

        def new_entry_writes(h, kt8, v8):
            # column/row for l == pos from the biased kvrow
            gp = pos % 1024
            nc.sync.dma_start(kt8[:, gp:gp + 1],
                              kvrow[0:1, h * DK:(h + 1) * DK])
            nc.sync.dma_start(
                v8[gp % 128:gp % 128 + 1, (gp // 128) * 128:(gp // 128 + 1) * 128],
                kvrow[0:1, HP * DK + h * DK:HP * DK + (h + 1) * DK])

        npos_g = pos // 1024            # l-group holding the new entry
        npos_lt = pos // 128            # l-tile index holding the new entry
        # riding is only possible when the S loop is long enough for the
        # 2-instruction-per-lt passes to finish before the new entry is used
        ride_kv = LC >= DC and npos_lt >= 8
        ride_q = LC >= DC

        # ---------- head 0 Q projection (phase A, DMA-paced) ----------
        bq_t = ss_pool.tile([128, 1], F32, name="bq0", tag="bq", bufs=2)
        nc.sync.dma_start(bq_t[:], bq_d[0])
        qT_t = qT_pool.tile([128, S], F32R, name="qT0", tag="qT")
        for half in range(2):
            psq_t = psq.tile([128, 512], F32, name=f"psq0_{half}", tag="psq")
            for c in range(DC):
                q_half_mm(psq_t, wq0_groups[c // 4], c, half)
            q_half_add(0, qT_t, psq_t, half, bq_t)

        if not ride_kv:
            # fallback: dense kv_new before the S loops
            for which in range(2):
                kv_t = kv_pool.tile([1, HP * DK], F32, name=f"kv{which}", tag="kv")
                for c in range(DC):
                    kv_mm(kv_t, c, which)
                kv_add(kv_t, which)

        for h in range(HP):
            # per-lt ride items emitted right after the scores matmuls
            rides = [[] for _ in range(LC)]
            if h + 1 < HP and ride_q:
                bq1 = ss_pool.tile([128, 1], F32, name=f"bq{h+1}", tag="bq",
                                   bufs=2)
                nc.sync.dma_start(bq1[:], bq_d[h + 1])
                qT_next = qT_pool.tile([128, S], F32R, name=f"qT{h+1}", tag="qT")
                state = {}

                def mk_q(lt, h1=h + 1, qn=qT_next, bqt=bq1, st=state):
                    def emit():
                        half, c0 = divmod(2 * lt, DC)
                        if c0 == 0:
                            st["psq"] = psq.tile([128, 512], F32,
                                                 name=f"psq{h1}_{half}", tag="psq")
                        for c in (c0, c0 + 1):
                            if c % 4 == 0:
                                st["wq"] = emit_wq_dma(h1, c // 4)
                            q_half_mm(st["psq"], st["wq"], c, half)
                        if c0 + 1 == DC - 1:
                            q_half_add(h1, qn, st["psq"], half, bqt)
                    return emit

                for lt in range(DC):
                    rides[lt].append(mk_q(lt))
            if h == 0 and ride_kv:
                # kv_new work items, paced so both passes (and their kvrow
                # writes) are emitted strictly before lt == npos_lt
                kv_work = ([("mm", 0, c) for c in range(DC)] + [("add", 0, 0)]
                           + [("mm", 1, c) for c in range(DC)] + [("add", 1, 0)])
                kvstate = {}

                def kv_emit_one(item, st=kvstate):
                    kind, which, c = item
                    if kind == "add":
                        kv_add(st["kv"], which)
                        return
                    if c == 0:
                        st["kv"] = kv_pool.tile([1, HP * DK], F32,
                                                name=f"kv{which}", tag="kv")
                    kv_mm(st["kv"], c, which)

                n_slots = npos_lt - 1          # ride slots: lt 0..npos_lt-2
                n_pre = max(0, len(kv_work) - 2 * n_slots)
                for item in kv_work[:n_pre]:
                    kv_emit_one(item)
                rest = kv_work[n_pre:]
                for k, item in enumerate(rest):
                    rides[k // 2].append(
                        (lambda it=item: kv_emit_one(it)))

            psc_t = psc.tile([128, S], F32, name=f"psc{h}", tag="psc")
            cur = load_group(h, 0)
            if not (h == 0 and ride_kv) and npos_g == 0 and npos_lt < LC:
                new_entry_writes(h, *cur)
            nxt = None
            pend = None              # lag-1 ctx: (lt, wt, vst)
            for lt in range(LC):
                g, j = lt // 8, lt % 8
                if j == 0 and g > 0:
                    cur = nxt
                if j == 0 and g + 1 < (LC + 7) // 8:
                    nxt = load_group(h, g + 1)
                    if not (h == 0 and ride_kv) and npos_g == g + 1:
                        new_entry_writes(h, *nxt)
                kt8, v8 = cur
                if h == 0 and ride_kv and lt == npos_lt:
                    # kvrow writes were emitted at lt <= npos_lt - 1
                    new_entry_writes(h, kt8, v8) if npos_g == g else None
                    if npos_g == g + 1 and nxt is not None:
                        new_entry_writes(h, *nxt)

                ps = pss.tile([128, 1024], F32, name=f"ps_{h}_{lt}", tag="pss")
                ksl = kt8[:, j * 128:(j + 1) * 128]
                nc.tensor.matmul(ps[:, 0:512], ksl, qT_t[:, 0:512])
                nc.tensor.matmul(ps[:, 512:1024], ksl, qT_t[:, 512:1024])

                for emit in rides[lt]:
                    emit()

                wt = wt_pool.tile([128, 1024], F32R, name=f"wt_{h}_{lt}", tag="wt")
                ssum = ss_pool.tile([128, 1], F32, name=f"ss_{h}_{lt}", tag="ssum")
                nc.scalar.activation(wt[:], ps[:], AF.Exp, scale=INV, accum_out=ssum[:])
                rec = ss_pool.tile([128, 1], F32, name=f"rc_{h}_{lt}", tag="rec")
                nc.vector.reciprocal(rec[:], ssum[:])
                vst = vs_pool.tile([128, DK], F32R, name=f"vs{h}_{lt}", tag="vs")
                nc.vector.tensor_scalar_mul(vst[:], v8[:, j * 128:(j + 1) * 128], rec[:])

                if pend is not None:
                    plt, pwt, pvst = pend
                    nc.tensor.matmul(psc_t[:, 0:512], pvst[:], pwt[:, 0:512],
                                     start=(plt == 0), stop=False)
                    nc.tensor.matmul(psc_t[:, 512:1024], pvst[:], pwt[:, 512:1024],
                                     start=(plt == 0), stop=False)
                pend = (lt, wt, vst)
            plt, pwt, pvst = pend
            nc.tensor.matmul(psc_t[:, 0:512], pvst[:], pwt[:, 0:512],
                             start=(plt == 0), stop=True)
            nc.tensor.matmul(psc_t[:, 512:1024], pvst[:], pwt[:, 512:1024],
                             start=(plt == 0), stop=True)
            nc.vector.tensor_copy(ctxTs[h][:], psc_t[:])
            if h + 1 < HP and not ride_q:
                # dense fallback Q projection for the next head
                bq1 = ss_pool.tile([128, 1], F32, name=f"bq{h+1}", tag="bq",
                                   bufs=2)
                nc.sync.dma_start(bq1[:], bq_d[h + 1])
                qT_next = qT_pool.tile([128, S], F32R, name=f"qT{h+1}", tag="qT")
                for half in range(2):
                    psq_t = psq.tile([128, 512], F32,
                                     name=f"psq{h+1}_{half}", tag="psq")
                    wqt = None
                    for c in range(DC):
                        if c % 4 == 0:
                            wqt = emit_wq_dma(h + 1, c // 4)
                        q_half_mm(psq_t, wqt, c, half)
                    q_half_add(h + 1, qT_next, psq_t, half, bq1)
            if h + 1 < HP:
                qT_t = qT_next

        # release attention-phase pools before the output projection (LIFO)
        for p in (psc, pss, kv_pool, psq,
                  ss_pool, vs_pool, wt_pool, v_pool, kt_pool,
                  wkv_pool, wq_pool, small, qT_pool, xT_pool):
            p.release()

        # ---------- output projection: out[s, m] partial ----------
        # Wo fully resident in the space freed by xT; one 16KB-burst output
        # DMA per s-tile.
        wo_pool = tc.alloc_tile_pool(name="wop", bufs=1)
        ob_pool = tc.alloc_tile_pool(name="obp", bufs=2)
        pso = tc.alloc_tile_pool(name="pso", bufs=4, space="PSUM")
        wos = []
        for c in range(HP):
            wot = wo_pool.tile([128, D], F32R, name=f"wo{c}", tag=f"wo{c}")
            nc.sync.dma_start(wot[:], wo_d[c * 128:(c + 1) * 128, :])
            wos.append(wot)
        for s_t in range(S // 128):
            ob = ob_pool.tile([128, D], F32, name=f"ob{s_t}", tag="ob")
            for mg in range(D // 512):
                pso_t = pso.tile([128, 512], F32, name=f"po{s_t}_{mg}", tag="pso")
                for c in range(HP):
                    nc.tensor.matmul(pso_t[:],
                                     ctxTs[c][:, s_t * 128:(s_t + 1) * 128],
                                     wos[c][:, mg * 512:(mg + 1) * 512],
                                     start=(c == 0), stop=(c == HP - 1))
                nc.vector.tensor_copy(ob[:, mg * 512:(mg + 1) * 512], pso_t[:])
            nc.sync.dma_start(out_d[s_t * 128:(s_t + 1) * 128, :], ob[:])
        for p in (pso, ob_pool, wo_pool, ctxT_pool):
            p.release()

    nc.compile()
    return nc


_CACHE = {}
LAST_EXEC_NS = None


def kernel(x, k_cache, v_cache, Wq, bq, Wk, bk, Wv, bv, Wo, bo, pos):
    global LAST_EXEC_NS
    pos = int(pos)

    def f32(a):
        return np.ascontiguousarray(np.asarray(a), dtype=np.float32)

    x = f32(x)
    k_cache, v_cache = f32(k_cache), f32(v_cache)
    Wq, Wk, Wv, Wo = f32(Wq), f32(Wk), f32(Wv), f32(Wo)
    bq, bk, bv, bo = f32(bq), f32(bk), f32(bv), f32(bo)

    xT = np.ascontiguousarray(x[0].T)              # [D, S]
    xl = np.ascontiguousarray(
        x[0, -1].reshape(DC, 128).T.astype(ml_dtypes.bfloat16))
    in_maps = []
    for i in range(NCORES):
        hs = slice(i * HP, (i + 1) * HP)
        in_maps.append({
            "xT": xT,
            "wq": np.ascontiguousarray(Wq[hs]),
            "wkv": np.ascontiguousarray(np.concatenate([
                Wk[hs].transpose(1, 0, 2).reshape(D, HP * DK),
                Wv[hs].transpose(1, 0, 2).reshape(D, HP * DK)],
                axis=1).astype(ml_dtypes.bfloat16)),
            "xl": xl,
            "bq": np.ascontiguousarray(bq[hs].reshape(HP, DK, 1)),
            "bkv": np.ascontiguousarray(np.concatenate(
                [bk[hs].reshape(-1), bv[hs].reshape(-1)])[None, :]),
            "kT": np.ascontiguousarray(k_cache[hs, :pos, :].transpose(0, 2, 1)),
            "v": np.ascontiguousarray(v_cache[hs, :pos, :]),
            "wo": np.ascontiguousarray(Wo[i * HP * DK:(i + 1) * HP * DK]),
        })

    if pos not in _CACHE:
        _CACHE[pos] = build(pos)
    nc = _CACHE[pos]

    res = run_bass_kernel_spmd(nc, in_maps, core_ids=list(range(NCORES)))
    LAST_EXEC_NS = res.exec_time_ns

    acc = np.zeros((S, D), np.float64)
    for r in res.results:
        acc += r["out"]
    out = (acc + bo.astype(np.float64)).astype(np.float32)
    return out[None]


# revision 20
# speedup vs baseline: 1.1812x; 1.0926x over previous
"""Trainium2 Bass kernel for CachedMultiHeadedAttention (tensor-parallel over heads).

Sharding: 8 cores x 4 heads. Each core computes Q projection + attention for
its 4 heads, then a partial output projection against its 512 rows of Wo.
Host sums the 8 partial outputs (the "all-reduce" done at unshard time) and
adds bo.

Device-side layouts are chosen so NO on-chip transposes are needed:
  - x is passed pre-transposed (xT [D, S]) so contraction dims land on
    SBUF partitions for every matmul.
  - k_cache is passed pre-transposed per head (kT [DK, pos]).
  - The reference's softmax quirk (softmax over the QUERY axis) maps to
    scores^T tiles [l_part, s_free]: one fused ACT pass does exp + row-sum.
    The 1/sum normalization is folded into V rows (8x less data than the
    weight matrix).
Matmuls run as float32r (full PE rate; plain fp32 matmul is 1/4 rate).
The k_new/v_new projections (rank-1 work) run in bf16 — they only affect one
of the 4096 cache rows, so their rounding is negligible in the output.

DMAs are consolidated into few large transfers: each dma_start costs ~625ns
on the single HWDGE queue, which was the dominant bottleneck of the naive
version (557 DMAs = 348us of serialized HWDGE time).
"""

import math

import numpy as np
import ml_dtypes

import concourse.bass as bass
import concourse.mybir as mybir
import concourse.tile as tile
from concourse import bacc
from concourse.bass_utils import run_bass_kernel_spmd

F32 = mybir.dt.float32
F32R = mybir.dt.float32r
BF16 = mybir.dt.bfloat16
F16 = mybir.dt.float16
AF = mybir.ActivationFunctionType

H, D, DK, S = 32, 4096, 128, 1024
NCORES = 8
HP = H // NCORES          # heads per core
DC = D // 128             # contraction chunks for d_model


def build(pos: int):
    L = pos + 1
    LC = (L + 127) // 128          # number of 128-wide l tiles
    LG = (LC + 7) // 8             # l-tile groups of 8 (1024 l per group)
    INV = 1.0 / math.sqrt(DK)

    nc = bacc.Bacc("TRN2", target_bir_lowering=False, debug=False,
                   num_devices=NCORES)

    xT_d = nc.dram_tensor("xT", [D, S], F16, kind="ExternalInput").ap()
    wq_d = nc.dram_tensor("wq", [HP, D, DK], F16, kind="ExternalInput").ap()
    wkv_d = nc.dram_tensor("wkv", [D, 2 * HP * DK], BF16, kind="ExternalInput").ap()
    xl_d = nc.dram_tensor("xl", [128, DC], BF16, kind="ExternalInput").ap()
    bq_d = nc.dram_tensor("bq", [HP, DK, 1], F32, kind="ExternalInput").ap()
    bkv_d = nc.dram_tensor("bkv", [1, 2 * HP * DK], F32, kind="ExternalInput").ap()
    kT_d = nc.dram_tensor("kT", [HP, DK, pos], F16, kind="ExternalInput").ap()
    v_d = nc.dram_tensor("v", [HP, pos, DK], F16, kind="ExternalInput").ap()
    wo_d = nc.dram_tensor("wo", [HP * DK, D], F16, kind="ExternalInput").ap()
    out_d = nc.dram_tensor("out", [S, D], F32, kind="ExternalOutput").ap()

    with tile.TileContext(nc) as tc:
        # Pools are released LIFO; ctxT survives into the output projection,
        # so it sits at the bottom of the SBUF pool stack.
        ctxT_pool = tc.alloc_tile_pool(name="ctxT", bufs=1)
        xT_pool = tc.alloc_tile_pool(name="xT", bufs=1)
        qT_pool = tc.alloc_tile_pool(name="qT", bufs=2)
        small = tc.alloc_tile_pool(name="smallp", bufs=1)
        wq_pool = tc.alloc_tile_pool(name="wqp", bufs=8)
        wkv_pool = tc.alloc_tile_pool(name="wkvp", bufs=3)
        kt_pool = tc.alloc_tile_pool(name="ktp", bufs=3)
        v_pool = tc.alloc_tile_pool(name="vp", bufs=3)
        wt_pool = tc.alloc_tile_pool(name="wtp", bufs=4)
        vs_pool = tc.alloc_tile_pool(name="vsp", bufs=4)
        ss_pool = tc.alloc_tile_pool(name="ssp", bufs=8)

        # PSUM budget (8 banks): psq 1 + kv 1 + pss 4 + psc 2.
        # Q projections and the kv_new projections run in TWO s-half /
        # k-v passes so their accumulators are single-bank.
        psq = tc.alloc_tile_pool(name="psq", bufs=1, space="PSUM")
        kv_pool = tc.alloc_tile_pool(name="kvp", bufs=1, space="PSUM")
        pss = tc.alloc_tile_pool(name="pss", bufs=2, space="PSUM")
        psc = tc.alloc_tile_pool(name="psc", bufs=1, space="PSUM")

        ctxTs = [ctxT_pool.tile([128, S], F16, name=f"cT{h}", tag=f"cT{h}")
                 for h in range(HP)]

        # small constants first (tiny DMAs, ahead of the big streams)
        kvrow = small.tile([1, 2 * HP * DK], F16, name="kvrow", tag="kvrow")
        bkv_t = small.tile([1, 2 * HP * DK], F32, name="bkvt", tag="bkvt")
        nc.sync.dma_start(bkv_t[:], bkv_d[:])
        xl_t = small.tile([128, DC], BF16, name="xlt", tag="xlt")
        nc.sync.dma_start(xl_t[:], xl_d[:])

        # resident xT tiles (8 big tiles of 4 chunks), interleaved with head
        # 0's Q weight groups so the first Q matmuls start after ~2.5MB, not
        # after the full 17MB of x.
        xbig = []
        wq0_groups = []
        for gx in range(DC // 4):
            wqt = wq_pool.tile([128, 4 * DK], F16, name=f"wq0_{gx}", tag="wq")
            nc.sync.dma_start(
                wqt[:], wq_d[0, gx * 512:(gx + 1) * 512, :].rearrange(
                    "(i p) k -> p i k", p=128))
            wq0_groups.append(wqt)
            xt = xT_pool.tile([128, 4 * S], F16, name=f"xt{gx}", tag=f"xt{gx}")
            nc.sync.dma_start(
                xt[:], xT_d[gx * 512:(gx + 1) * 512, :].rearrange(
                    "(i p) s -> p i s", p=128))
            xbig.append(xt)

        def xsl(c, lo, sz):
            return xbig[c // 4][:, (c % 4) * S + lo:(c % 4) * S + lo + sz]

        def emit_wq_dma(h, gw, tag="wq"):
            wqt = wq_pool.tile([128, 4 * DK], F16,
                               name=f"wq{h}_{gw}", tag=tag)
            nc.sync.dma_start(
                wqt[:], wq_d[h, gw * 512:(gw + 1) * 512, :].rearrange(
                    "(i p) k -> p i k", p=128))
            return wqt

        def q_half_mm(psq_t, wqt, c, half):
            lhs = wqt[:, (c % 4) * DK:(c % 4 + 1) * DK]
            nc.tensor.matmul(psq_t[:], lhs, xsl(c, half * 512, 512),
                             start=(c == 0), stop=(c == DC - 1))

        def q_half_add(h, qT_t, psq_t, half, bq_t):
            nc.vector.tensor_scalar_add(qT_t[:, half * 512:(half + 1) * 512],
                                        psq_t[:], bq_t[:])

        def kv_mm(kv_t, c, which):
            # which: 0 = k_new, 1 = v_new
            wkvt = wkv_pool.tile([128, HP * DK], BF16,
                                 name=f"wkv{which}_{c}", tag="wkv")
            nc.sync.dma_start(
                wkvt[:], wkv_d[c * 128:(c + 1) * 128,
                               which * HP * DK:(which + 1) * HP * DK])
            nc.tensor.matmul(kv_t[0:1, :], xl_t[:, c:c + 1], wkvt[:],
                             start=(c == 0), stop=(c == DC - 1))

        def kv_add(kv_t, which):
            nc.vector.tensor_add(
                kvrow[0:1, which * HP * DK:(which + 1) * HP * DK], kv_t[:],
                bkv_t[0:1, which * HP * DK:(which + 1) * HP * DK])

        def load_group(h, g):
            """Cache-only loads of l-group g (the new-entry writes are
            emitted separately, after kvrow's writes in trace order)."""
            g0 = g * 1024
            gl = min(1024, L - g0)            # valid l in group
            gc = max(0, min(1024, pos - g0))  # of which from cache
            kt8 = kt_pool.tile([128, 1024], F16, name=f"kt{h}_{g}", tag="kt")
            if gc > 0:
                nc.sync.dma_start(kt8[:, 0:gc], kT_d[h, :, g0:g0 + gc])
            if gl < 1024:
                nc.vector.memset(kt8[:, gl:1024], 0.0)
            v8 = v_pool.tile([128, 1024], F16, name=f"v{h}_{g}", tag="v")
            fc = gc // 128
            if fc > 0:
                nc.sync.dma_start(
                    v8[:, 0:fc * 128],
                    v_d[h, g0:g0 + fc * 128, :].rearrange(
                        "(i p) k -> p i k", p=128))
            rem = gc - fc * 128
            if rem > 0:
                nc.sync.dma_start(v8[0:rem, fc * 128:(fc + 1) * 128],
                                  v_d[h, g0 + fc * 128:g0 + gc, :])
            if gl < 1024:
                for cc in range(gl // 128, 8):
                    lo = max(gl - cc * 128, 0)
                    if lo < 128:
                        nc.vector.memset(v8[lo:128, cc * 128:(cc + 1) * 128], 0.0)
            return kt8, v8

        def new_entry_writes(h, kt8, v8):
            # column/row for l == pos from the biased kvrow
            gp = pos % 1024
            nc.sync.dma_start(kt8[:, gp:gp + 1],
                              kvrow[0:1, h * DK:(h + 1) * DK])
            nc.sync.dma_start(
                v8[gp % 128:gp % 128 + 1, (gp // 128) * 128:(gp // 128 + 1) * 128],
                kvrow[0:1, HP * DK + h * DK:HP * DK + (h + 1) * DK])

        npos_g = pos // 1024            # l-group holding the new entry
        npos_lt = pos // 128            # l-tile index holding the new entry
        # riding is only possible when the S loop is long enough for the
        # 2-instruction-per-lt passes to finish before the new entry is used
        ride_kv = LC >= DC and npos_lt >= 8
        ride_q = LC >= DC

        # ---------- head 0 Q projection (phase A, DMA-paced) ----------
        bq_t = ss_pool.tile([128, 1], F32, name="bq0", tag="bq", bufs=2)
        nc.sync.dma_start(bq_t[:], bq_d[0])
        qT_t = qT_pool.tile([128, S], F16, name="qT0", tag="qT")
        for half in range(2):
            psq_t = psq.tile([128, 512], F32, name=f"psq0_{half}", tag="psq")
            for c in range(DC):
                q_half_mm(psq_t, wq0_groups[c // 4], c, half)
            q_half_add(0, qT_t, psq_t, half, bq_t)

        if not ride_kv:
            # fallback: dense kv_new before the S loops
            for which in range(2):
                kv_t = kv_pool.tile([1, HP * DK], F32, name=f"kv{which}", tag="kv")
                for c in range(DC):
                    kv_mm(kv_t, c, which)
                kv_add(kv_t, which)

        for h in range(HP):
            # per-lt ride items emitted right after the scores matmuls
            rides = [[] for _ in range(LC)]
            if h + 1 < HP and ride_q:
                bq1 = ss_pool.tile([128, 1], F32, name=f"bq{h+1}", tag="bq",
                                   bufs=2)
                nc.sync.dma_start(bq1[:], bq_d[h + 1])
                qT_next = qT_pool.tile([128, S], F16, name=f"qT{h+1}", tag="qT")
                state = {}

                def mk_q(lt, h1=h + 1, qn=qT_next, bqt=bq1, st=state):
                    def emit():
                        half, c0 = divmod(2 * lt, DC)
                        if c0 == 0:
                            st["psq"] = psq.tile([128, 512], F32,
                                                 name=f"psq{h1}_{half}", tag="psq")
                        for c in (c0, c0 + 1):
                            if c % 4 == 0:
                                st["wq"] = emit_wq_dma(h1, c // 4)
                            q_half_mm(st["psq"], st["wq"], c, half)
                        if c0 + 1 == DC - 1:
                            q_half_add(h1, qn, st["psq"], half, bqt)
                    return emit

                for lt in range(DC):
                    rides[lt].append(mk_q(lt))
            if h == 0 and ride_kv:
                # kv_new work items, paced so both passes (and their kvrow
                # writes) are emitted strictly before lt == npos_lt
                kv_work = ([("mm", 0, c) for c in range(DC)] + [("add", 0, 0)]
                           + [("mm", 1, c) for c in range(DC)] + [("add", 1, 0)])
                kvstate = {}

                def kv_emit_one(item, st=kvstate):
                    kind, which, c = item
                    if kind == "add":
                        kv_add(st["kv"], which)
                        return
                    if c == 0:
                        st["kv"] = kv_pool.tile([1, HP * DK], F32,
                                                name=f"kv{which}", tag="kv")
                    kv_mm(st["kv"], c, which)

                n_slots = npos_lt - 1          # ride slots: lt 0..npos_lt-2
                n_pre = max(0, len(kv_work) - 2 * n_slots)
                for item in kv_work[:n_pre]:
                    kv_emit_one(item)
                rest = kv_work[n_pre:]
                for k, item in enumerate(rest):
                    rides[k // 2].append(
                        (lambda it=item: kv_emit_one(it)))

            psc_t = psc.tile([128, S], F32, name=f"psc{h}", tag="psc")
            cur = load_group(h, 0)
            if not (h == 0 and ride_kv) and npos_g == 0 and npos_lt < LC:
                new_entry_writes(h, *cur)
            nxt = None
            pend = None              # lag-1 ctx: (lt, wt, vst)
            for lt in range(LC):
                g, j = lt // 8, lt % 8
                if j == 0 and g > 0:
                    cur = nxt
                if j == 0 and g + 1 < (LC + 7) // 8:
                    nxt = load_group(h, g + 1)
                    if not (h == 0 and ride_kv) and npos_g == g + 1:
                        new_entry_writes(h, *nxt)
                kt8, v8 = cur
                if h == 0 and ride_kv and lt == npos_lt:
                    # kvrow writes were emitted at lt <= npos_lt - 1
                    new_entry_writes(h, kt8, v8) if npos_g == g else None
                    if npos_g == g + 1 and nxt is not None:
                        new_entry_writes(h, *nxt)

                ps = pss.tile([128, 1024], F32, name=f"ps_{h}_{lt}", tag="pss")
                ksl = kt8[:, j * 128:(j + 1) * 128]
                nc.tensor.matmul(ps[:, 0:512], ksl, qT_t[:, 0:512])
                nc.tensor.matmul(ps[:, 512:1024], ksl, qT_t[:, 512:1024])

                for emit in rides[lt]:
                    emit()

                wt = wt_pool.tile([128, 1024], F32R, name=f"wt_{h}_{lt}", tag="wt")
                ssum = ss_pool.tile([128, 1], F32, name=f"ss_{h}_{lt}", tag="ssum")
                nc.scalar.activation(wt[:], ps[:], AF.Exp, scale=INV, accum_out=ssum[:])
                rec = ss_pool.tile([128, 1], F32, name=f"rc_{h}_{lt}", tag="rec")
                nc.vector.reciprocal(rec[:], ssum[:])
                vst = vs_pool.tile([128, DK], F32R, name=f"vs{h}_{lt}", tag="vs")
                nc.vector.tensor_scalar_mul(vst[:], v8[:, j * 128:(j + 1) * 128], rec[:])

                if pend is not None:
                    plt, pwt, pvst = pend
                    nc.tensor.matmul(psc_t[:, 0:512], pvst[:], pwt[:, 0:512],
                                     start=(plt == 0), stop=False)
                    nc.tensor.matmul(psc_t[:, 512:1024], pvst[:], pwt[:, 512:1024],
                                     start=(plt == 0), stop=False)
                pend = (lt, wt, vst)
            plt, pwt, pvst = pend
            nc.tensor.matmul(psc_t[:, 0:512], pvst[:], pwt[:, 0:512],
                             start=(plt == 0), stop=True)
            nc.tensor.matmul(psc_t[:, 512:1024], pvst[:], pwt[:, 512:1024],
                             start=(plt == 0), stop=True)
            nc.vector.tensor_copy(ctxTs[h][:], psc_t[:])
            if h + 1 < HP and not ride_q:
                # dense fallback Q projection for the next head
                bq1 = ss_pool.tile([128, 1], F32, name=f"bq{h+1}", tag="bq",
                                   bufs=2)
                nc.sync.dma_start(bq1[:], bq_d[h + 1])
                qT_next = qT_pool.tile([128, S], F16, name=f"qT{h+1}", tag="qT")
                for half in range(2):
                    psq_t = psq.tile([128, 512], F32,
                                     name=f"psq{h+1}_{half}", tag="psq")
                    wqt = None
                    for c in range(DC):
                        if c % 4 == 0:
                            wqt = emit_wq_dma(h + 1, c // 4)
                        q_half_mm(psq_t, wqt, c, half)
                    q_half_add(h + 1, qT_next, psq_t, half, bq1)
            if h + 1 < HP:
                qT_t = qT_next

        # release attention-phase pools before the output projection (LIFO)
        for p in (psc, pss, kv_pool, psq,
                  ss_pool, vs_pool, wt_pool, v_pool, kt_pool,
                  wkv_pool, wq_pool, small, qT_pool, xT_pool):
            p.release()

        # ---------- output projection: out[s, m] partial ----------
        # Wo fully resident in the space freed by xT; one 16KB-burst output
        # DMA per s-tile.
        wo_pool = tc.alloc_tile_pool(name="wop", bufs=1)
        ob_pool = tc.alloc_tile_pool(name="obp", bufs=2)
        pso = tc.alloc_tile_pool(name="pso", bufs=4, space="PSUM")
        wos = []
        for c in range(HP):
            wot = wo_pool.tile([128, D], F16, name=f"wo{c}", tag=f"wo{c}")
            nc.sync.dma_start(wot[:], wo_d[c * 128:(c + 1) * 128, :])
            wos.append(wot)
        for s_t in range(S // 128):
            ob = ob_pool.tile([128, D], F32, name=f"ob{s_t}", tag="ob")
            for mg in range(D // 512):
                pso_t = pso.tile([128, 512], F32, name=f"po{s_t}_{mg}", tag="pso")
                for c in range(HP):
                    nc.tensor.matmul(pso_t[:],
                                     ctxTs[c][:, s_t * 128:(s_t + 1) * 128],
                                     wos[c][:, mg * 512:(mg + 1) * 512],
                                     start=(c == 0), stop=(c == HP - 1))
                nc.vector.tensor_copy(ob[:, mg * 512:(mg + 1) * 512], pso_t[:])
            nc.sync.dma_start(out_d[s_t * 128:(s_t + 1) * 128, :], ob[:])
        for p in (pso, ob_pool, wo_pool, ctxT_pool):
            p.release()

    nc.compile()
    return nc


_CACHE = {}
LAST_EXEC_NS = None


def kernel(x, k_cache, v_cache, Wq, bq, Wk, bk, Wv, bv, Wo, bo, pos):
    global LAST_EXEC_NS
    pos = int(pos)

    def f32(a):
        return np.ascontiguousarray(np.asarray(a), dtype=np.float32)

    x = f32(x)
    k_cache, v_cache = f32(k_cache), f32(v_cache)
    Wq, Wk, Wv, Wo = f32(Wq), f32(Wk), f32(Wv), f32(Wo)
    bq, bk, bv, bo = f32(bq), f32(bk), f32(bv), f32(bo)

    xT = np.ascontiguousarray(x[0].T.astype(np.float16))   # [D, S]
    xl = np.ascontiguousarray(
        x[0, -1].reshape(DC, 128).T.astype(ml_dtypes.bfloat16))
    in_maps = []
    for i in range(NCORES):
        hs = slice(i * HP, (i + 1) * HP)
        in_maps.append({
            "xT": xT,
            "wq": np.ascontiguousarray(Wq[hs].astype(np.float16)),
            "wkv": np.ascontiguousarray(np.concatenate([
                Wk[hs].transpose(1, 0, 2).reshape(D, HP * DK),
                Wv[hs].transpose(1, 0, 2).reshape(D, HP * DK)],
                axis=1).astype(ml_dtypes.bfloat16)),
            "xl": xl,
            "bq": np.ascontiguousarray(bq[hs].reshape(HP, DK, 1)),
            "bkv": np.ascontiguousarray(np.concatenate(
                [bk[hs].reshape(-1), bv[hs].reshape(-1)])[None, :]),
            "kT": np.ascontiguousarray(
                k_cache[hs, :pos, :].transpose(0, 2, 1).astype(np.float16)),
            "v": np.ascontiguousarray(v_cache[hs, :pos, :].astype(np.float16)),
            "wo": np.ascontiguousarray(
                Wo[i * HP * DK:(i + 1) * HP * DK].astype(np.float16)),
        })

    if pos not in _CACHE:
        _CACHE[pos] = build(pos)
    nc = _CACHE[pos]

    res = run_bass_kernel_spmd(nc, in_maps, core_ids=list(range(NCORES)))
    LAST_EXEC_NS = res.exec_time_ns

    acc = np.zeros((S, D), np.float64)
    for r in res.results:
        acc += r["out"]
    out = (acc + bo.astype(np.float64)).astype(np.float32)
    return out[None]


# revision 21
# speedup vs baseline: 1.1929x; 1.0099x over previous
"""Trainium2 Bass kernel for CachedMultiHeadedAttention (tensor-parallel over heads).

Sharding: 8 cores x 4 heads. Each core computes Q projection + attention for
its 4 heads, then a partial output projection against its 512 rows of Wo.
Host sums the 8 partial outputs (the "all-reduce" done at unshard time) and
adds bo.

Device-side layouts are chosen so NO on-chip transposes are needed:
  - x is passed pre-transposed (xT [D, S]) so contraction dims land on
    SBUF partitions for every matmul.
  - k_cache is passed pre-transposed per head (kT [DK, pos]).
  - The reference's softmax quirk (softmax over the QUERY axis) maps to
    scores^T tiles [l_part, s_free]: one fused ACT pass does exp + row-sum.
    The 1/sum normalization is folded into V rows (8x less data than the
    weight matrix).
Matmuls run as float32r (full PE rate; plain fp32 matmul is 1/4 rate).
The k_new/v_new projections (rank-1 work) run in bf16 — they only affect one
of the 4096 cache rows, so their rounding is negligible in the output.

DMAs are consolidated into few large transfers: each dma_start costs ~625ns
on the single HWDGE queue, which was the dominant bottleneck of the naive
version (557 DMAs = 348us of serialized HWDGE time).
"""

import math

import numpy as np
import ml_dtypes

import concourse.bass as bass
import concourse.mybir as mybir
import concourse.tile as tile
from concourse import bacc
from concourse.bass_utils import run_bass_kernel_spmd

F32 = mybir.dt.float32
F32R = mybir.dt.float32r
BF16 = mybir.dt.bfloat16
F16 = mybir.dt.float16
AF = mybir.ActivationFunctionType

H, D, DK, S = 32, 4096, 128, 1024
NCORES = 8
HP = H // NCORES          # heads per core
DC = D // 128             # contraction chunks for d_model


def build(pos: int):
    L = pos + 1
    LC = (L + 127) // 128          # number of 128-wide l tiles
    LG = (LC + 7) // 8             # l-tile groups of 8 (1024 l per group)
    INV = 1.0 / math.sqrt(DK)

    nc = bacc.Bacc("TRN2", target_bir_lowering=False, debug=False,
                   num_devices=NCORES)

    xT_d = nc.dram_tensor("xT", [D, S], F16, kind="ExternalInput").ap()
    wq_d = nc.dram_tensor("wq", [HP, D, DK], F16, kind="ExternalInput").ap()
    wkv_d = nc.dram_tensor("wkv", [D, 2 * HP * DK], BF16, kind="ExternalInput").ap()
    xl_d = nc.dram_tensor("xl", [128, DC], BF16, kind="ExternalInput").ap()
    bq_d = nc.dram_tensor("bq", [HP, DK, 1], F32, kind="ExternalInput").ap()
    bkv_d = nc.dram_tensor("bkv", [1, 2 * HP * DK], F32, kind="ExternalInput").ap()
    kT_d = nc.dram_tensor("kT", [HP, DK, pos], F16, kind="ExternalInput").ap()
    v_d = nc.dram_tensor("v", [HP, pos, DK], F16, kind="ExternalInput").ap()
    wo_d = nc.dram_tensor("wo", [HP * DK, D], F16, kind="ExternalInput").ap()
    out_d = nc.dram_tensor("out", [S, D], F16, kind="ExternalOutput").ap()

    with tile.TileContext(nc) as tc:
        # Pools are released LIFO; ctxT survives into the output projection,
        # so it sits at the bottom of the SBUF pool stack.
        ctxT_pool = tc.alloc_tile_pool(name="ctxT", bufs=1)
        xT_pool = tc.alloc_tile_pool(name="xT", bufs=1)
        qT_pool = tc.alloc_tile_pool(name="qT", bufs=2)
        small = tc.alloc_tile_pool(name="smallp", bufs=1)
        wq_pool = tc.alloc_tile_pool(name="wqp", bufs=8)
        wkv_pool = tc.alloc_tile_pool(name="wkvp", bufs=3)
        kt_pool = tc.alloc_tile_pool(name="ktp", bufs=3)
        v_pool = tc.alloc_tile_pool(name="vp", bufs=3)
        wt_pool = tc.alloc_tile_pool(name="wtp", bufs=4)
        vs_pool = tc.alloc_tile_pool(name="vsp", bufs=4)
        ss_pool = tc.alloc_tile_pool(name="ssp", bufs=8)

        # PSUM budget (8 banks): psq 1 + kv 1 + pss 4 + psc 2.
        # Q projections and the kv_new projections run in TWO s-half /
        # k-v passes so their accumulators are single-bank.
        psq = tc.alloc_tile_pool(name="psq", bufs=1, space="PSUM")
        kv_pool = tc.alloc_tile_pool(name="kvp", bufs=1, space="PSUM")
        pss = tc.alloc_tile_pool(name="pss", bufs=2, space="PSUM")
        psc = tc.alloc_tile_pool(name="psc", bufs=1, space="PSUM")

        ctxTs = [ctxT_pool.tile([128, S], F16, name=f"cT{h}", tag=f"cT{h}")
                 for h in range(HP)]

        # small constants first (tiny DMAs, ahead of the big streams)
        kvrow = small.tile([1, 2 * HP * DK], F16, name="kvrow", tag="kvrow")
        bkv_t = small.tile([1, 2 * HP * DK], F32, name="bkvt", tag="bkvt")
        nc.sync.dma_start(bkv_t[:], bkv_d[:])
        xl_t = small.tile([128, DC], BF16, name="xlt", tag="xlt")
        nc.sync.dma_start(xl_t[:], xl_d[:])

        # resident xT tiles (8 big tiles of 4 chunks), interleaved with head
        # 0's Q weight groups so the first Q matmuls start after ~2.5MB, not
        # after the full 17MB of x.
        xbig = []
        wq0_groups = []
        for gx in range(DC // 4):
            wqt = wq_pool.tile([128, 4 * DK], F16, name=f"wq0_{gx}", tag="wq")
            nc.sync.dma_start(
                wqt[:], wq_d[0, gx * 512:(gx + 1) * 512, :].rearrange(
                    "(i p) k -> p i k", p=128))
            wq0_groups.append(wqt)
            xt = xT_pool.tile([128, 4 * S], F16, name=f"xt{gx}", tag=f"xt{gx}")
            nc.sync.dma_start(
                xt[:], xT_d[gx * 512:(gx + 1) * 512, :].rearrange(
                    "(i p) s -> p i s", p=128))
            xbig.append(xt)

        def xsl(c, lo, sz):
            return xbig[c // 4][:, (c % 4) * S + lo:(c % 4) * S + lo + sz]

        def emit_wq_dma(h, gw, tag="wq"):
            wqt = wq_pool.tile([128, 4 * DK], F16,
                               name=f"wq{h}_{gw}", tag=tag)
            nc.sync.dma_start(
                wqt[:], wq_d[h, gw * 512:(gw + 1) * 512, :].rearrange(
                    "(i p) k -> p i k", p=128))
            return wqt

        def q_half_mm(psq_t, wqt, c, half):
            lhs = wqt[:, (c % 4) * DK:(c % 4 + 1) * DK]
            nc.tensor.matmul(psq_t[:], lhs, xsl(c, half * 512, 512),
                             start=(c == 0), stop=(c == DC - 1))

        def q_half_add(h, qT_t, psq_t, half, bq_t):
            nc.vector.tensor_scalar_add(qT_t[:, half * 512:(half + 1) * 512],
                                        psq_t[:], bq_t[:])

        def kv_mm(kv_t, c, which):
            # which: 0 = k_new, 1 = v_new
            wkvt = wkv_pool.tile([128, HP * DK], BF16,
                                 name=f"wkv{which}_{c}", tag="wkv")
            nc.sync.dma_start(
                wkvt[:], wkv_d[c * 128:(c + 1) * 128,
                               which * HP * DK:(which + 1) * HP * DK])
            nc.tensor.matmul(kv_t[0:1, :], xl_t[:, c:c + 1], wkvt[:],
                             start=(c == 0), stop=(c == DC - 1))

        def kv_add(kv_t, which):
            nc.vector.tensor_add(
                kvrow[0:1, which * HP * DK:(which + 1) * HP * DK], kv_t[:],
                bkv_t[0:1, which * HP * DK:(which + 1) * HP * DK])

        def load_group(h, g):
            """Cache-only loads of l-group g (the new-entry writes are
            emitted separately, after kvrow's writes in trace order)."""
            g0 = g * 1024
            gl = min(1024, L - g0)            # valid l in group
            gc = max(0, min(1024, pos - g0))  # of which from cache
            kt8 = kt_pool.tile([128, 1024], F16, name=f"kt{h}_{g}", tag="kt")
            if gc > 0:
                nc.sync.dma_start(kt8[:, 0:gc], kT_d[h, :, g0:g0 + gc])
            if gl < 1024:
                nc.vector.memset(kt8[:, gl:1024], 0.0)
            v8 = v_pool.tile([128, 1024], F16, name=f"v{h}_{g}", tag="v")
            fc = gc // 128
            if fc > 0:
                nc.sync.dma_start(
                    v8[:, 0:fc * 128],
                    v_d[h, g0:g0 + fc * 128, :].rearrange(
                        "(i p) k -> p i k", p=128))
            rem = gc - fc * 128
            if rem > 0:
                nc.sync.dma_start(v8[0:rem, fc * 128:(fc + 1) * 128],
                                  v_d[h, g0 + fc * 128:g0 + gc, :])
            if gl < 1024:
                for cc in range(gl // 128, 8):
                    lo = max(gl - cc * 128, 0)
                    if lo < 128:
                        nc.vector.memset(v8[lo:128, cc * 128:(cc + 1) * 128], 0.0)
            return kt8, v8

        def new_entry_writes(h, kt8, v8):
            # column/row for l == pos from the biased kvrow
            gp = pos % 1024
            nc.sync.dma_start(kt8[:, gp:gp + 1],
                              kvrow[0:1, h * DK:(h + 1) * DK])
            nc.sync.dma_start(
                v8[gp % 128:gp % 128 + 1, (gp // 128) * 128:(gp // 128 + 1) * 128],
                kvrow[0:1, HP * DK + h * DK:HP * DK + (h + 1) * DK])

        npos_g = pos // 1024            # l-group holding the new entry
        npos_lt = pos // 128            # l-tile index holding the new entry
        # riding is only possible when the S loop is long enough for the
        # 2-instruction-per-lt passes to finish before the new entry is used
        ride_kv = LC >= DC and npos_lt >= 8
        ride_q = LC >= DC

        # ---------- head 0 Q projection (phase A, DMA-paced) ----------
        bq_t = ss_pool.tile([128, 1], F32, name="bq0", tag="bq", bufs=2)
        nc.sync.dma_start(bq_t[:], bq_d[0])
        qT_t = qT_pool.tile([128, S], F16, name="qT0", tag="qT")
        for half in range(2):
            psq_t = psq.tile([128, 512], F32, name=f"psq0_{half}", tag="psq")
            for c in range(DC):
                q_half_mm(psq_t, wq0_groups[c // 4], c, half)
            q_half_add(0, qT_t, psq_t, half, bq_t)

        if not ride_kv:
            # fallback: dense kv_new before the S loops
            for which in range(2):
                kv_t = kv_pool.tile([1, HP * DK], F32, name=f"kv{which}", tag="kv")
                for c in range(DC):
                    kv_mm(kv_t, c, which)
                kv_add(kv_t, which)

        for h in range(HP):
            # per-lt ride items emitted right after the scores matmuls
            rides = [[] for _ in range(LC)]
            if h + 1 < HP and ride_q:
                bq1 = ss_pool.tile([128, 1], F32, name=f"bq{h+1}", tag="bq",
                                   bufs=2)
                nc.sync.dma_start(bq1[:], bq_d[h + 1])
                qT_next = qT_pool.tile([128, S], F16, name=f"qT{h+1}", tag="qT")
                state = {}

                def mk_q(lt, h1=h + 1, qn=qT_next, bqt=bq1, st=state):
                    def emit():
                        half, c0 = divmod(2 * lt, DC)
                        if c0 == 0:
                            st["psq"] = psq.tile([128, 512], F32,
                                                 name=f"psq{h1}_{half}", tag="psq")
                        for c in (c0, c0 + 1):
                            if c % 4 == 0:
                                st["wq"] = emit_wq_dma(h1, c // 4)
                            q_half_mm(st["psq"], st["wq"], c, half)
                        if c0 + 1 == DC - 1:
                            q_half_add(h1, qn, st["psq"], half, bqt)
                    return emit

                for lt in range(DC):
                    rides[lt].append(mk_q(lt))
            if h == 0 and ride_kv:
                # kv_new work items, paced so both passes (and their kvrow
                # writes) are emitted strictly before lt == npos_lt
                kv_work = ([("mm", 0, c) for c in range(DC)] + [("add", 0, 0)]
                           + [("mm", 1, c) for c in range(DC)] + [("add", 1, 0)])
                kvstate = {}

                def kv_emit_one(item, st=kvstate):
                    kind, which, c = item
                    if kind == "add":
                        kv_add(st["kv"], which)
                        return
                    if c == 0:
                        st["kv"] = kv_pool.tile([1, HP * DK], F32,
                                                name=f"kv{which}", tag="kv")
                    kv_mm(st["kv"], c, which)

                n_slots = npos_lt - 1          # ride slots: lt 0..npos_lt-2
                n_pre = max(0, len(kv_work) - 2 * n_slots)
                for item in kv_work[:n_pre]:
                    kv_emit_one(item)
                rest = kv_work[n_pre:]
                for k, item in enumerate(rest):
                    rides[k // 2].append(
                        (lambda it=item: kv_emit_one(it)))

            psc_t = psc.tile([128, S], F32, name=f"psc{h}", tag="psc")
            cur = load_group(h, 0)
            if not (h == 0 and ride_kv) and npos_g == 0 and npos_lt < LC:
                new_entry_writes(h, *cur)
            nxt = None
            pend = None              # lag-1 ctx: (lt, wt, vst)
            for lt in range(LC):
                g, j = lt // 8, lt % 8
                if j == 0 and g > 0:
                    cur = nxt
                if j == 0 and g + 1 < (LC + 7) // 8:
                    nxt = load_group(h, g + 1)
                    if not (h == 0 and ride_kv) and npos_g == g + 1:
                        new_entry_writes(h, *nxt)
                kt8, v8 = cur
                if h == 0 and ride_kv and lt == npos_lt:
                    # kvrow writes were emitted at lt <= npos_lt - 1
                    new_entry_writes(h, kt8, v8) if npos_g == g else None
                    if npos_g == g + 1 and nxt is not None:
                        new_entry_writes(h, *nxt)

                ps = pss.tile([128, 1024], F32, name=f"ps_{h}_{lt}", tag="pss")
                ksl = kt8[:, j * 128:(j + 1) * 128]
                nc.tensor.matmul(ps[:, 0:512], ksl, qT_t[:, 0:512])
                nc.tensor.matmul(ps[:, 512:1024], ksl, qT_t[:, 512:1024])

                for emit in rides[lt]:
                    emit()

                wt = wt_pool.tile([128, 1024], F32R, name=f"wt_{h}_{lt}", tag="wt")
                ssum = ss_pool.tile([128, 1], F32, name=f"ss_{h}_{lt}", tag="ssum")
                nc.scalar.activation(wt[:], ps[:], AF.Exp, scale=INV, accum_out=ssum[:])
                rec = ss_pool.tile([128, 1], F32, name=f"rc_{h}_{lt}", tag="rec")
                nc.vector.reciprocal(rec[:], ssum[:])
                vst = vs_pool.tile([128, DK], F32R, name=f"vs{h}_{lt}", tag="vs")
                nc.vector.tensor_scalar_mul(vst[:], v8[:, j * 128:(j + 1) * 128], rec[:])

                if pend is not None:
                    plt, pwt, pvst = pend
                    nc.tensor.matmul(psc_t[:, 0:512], pvst[:], pwt[:, 0:512],
                                     start=(plt == 0), stop=False)
                    nc.tensor.matmul(psc_t[:, 512:1024], pvst[:], pwt[:, 512:1024],
                                     start=(plt == 0), stop=False)
                pend = (lt, wt, vst)
            plt, pwt, pvst = pend
            nc.tensor.matmul(psc_t[:, 0:512], pvst[:], pwt[:, 0:512],
                             start=(plt == 0), stop=True)
            nc.tensor.matmul(psc_t[:, 512:1024], pvst[:], pwt[:, 512:1024],
                             start=(plt == 0), stop=True)
            nc.vector.tensor_copy(ctxTs[h][:], psc_t[:])
            if h + 1 < HP and not ride_q:
                # dense fallback Q projection for the next head
                bq1 = ss_pool.tile([128, 1], F32, name=f"bq{h+1}", tag="bq",
                                   bufs=2)
                nc.sync.dma_start(bq1[:], bq_d[h + 1])
                qT_next = qT_pool.tile([128, S], F16, name=f"qT{h+1}", tag="qT")
                for half in range(2):
                    psq_t = psq.tile([128, 512], F32,
                                     name=f"psq{h+1}_{half}", tag="psq")
                    wqt = None
                    for c in range(DC):
                        if c % 4 == 0:
                            wqt = emit_wq_dma(h + 1, c // 4)
                        q_half_mm(psq_t, wqt, c, half)
                    q_half_add(h + 1, qT_next, psq_t, half, bq1)
            if h + 1 < HP:
                qT_t = qT_next

        # release attention-phase pools before the output projection (LIFO)
        for p in (psc, pss, kv_pool, psq,
                  ss_pool, vs_pool, wt_pool, v_pool, kt_pool,
                  wkv_pool, wq_pool, small, qT_pool, xT_pool):
            p.release()

        # ---------- output projection: out[s, m] partial ----------
        # Wo fully resident in the space freed by xT; one 16KB-burst output
        # DMA per s-tile.
        wo_pool = tc.alloc_tile_pool(name="wop", bufs=1)
        ob_pool = tc.alloc_tile_pool(name="obp", bufs=2)
        pso = tc.alloc_tile_pool(name="pso", bufs=4, space="PSUM")
        wos = []
        for c in range(HP):
            wot = wo_pool.tile([128, D], F16, name=f"wo{c}", tag=f"wo{c}")
            nc.sync.dma_start(wot[:], wo_d[c * 128:(c + 1) * 128, :])
            wos.append(wot)
        for s_t in range(S // 128):
            ob = ob_pool.tile([128, D], F16, name=f"ob{s_t}", tag="ob")
            for mg in range(D // 512):
                pso_t = pso.tile([128, 512], F32, name=f"po{s_t}_{mg}", tag="pso")
                for c in range(HP):
                    nc.tensor.matmul(pso_t[:],
                                     ctxTs[c][:, s_t * 128:(s_t + 1) * 128],
                                     wos[c][:, mg * 512:(mg + 1) * 512],
                                     start=(c == 0), stop=(c == HP - 1))
                nc.vector.tensor_copy(ob[:, mg * 512:(mg + 1) * 512], pso_t[:])
            nc.sync.dma_start(out_d[s_t * 128:(s_t + 1) * 128, :], ob[:])
        for p in (pso, ob_pool, wo_pool, ctxT_pool):
            p.release()

    nc.compile()
    return nc


_CACHE = {}
LAST_EXEC_NS = None


def kernel(x, k_cache, v_cache, Wq, bq, Wk, bk, Wv, bv, Wo, bo, pos):
    global LAST_EXEC_NS
    pos = int(pos)

    def f32(a):
        return np.ascontiguousarray(np.asarray(a), dtype=np.float32)

    x = f32(x)
    k_cache, v_cache = f32(k_cache), f32(v_cache)
    Wq, Wk, Wv, Wo = f32(Wq), f32(Wk), f32(Wv), f32(Wo)
    bq, bk, bv, bo = f32(bq), f32(bk), f32(bv), f32(bo)

    xT = np.ascontiguousarray(x[0].T.astype(np.float16))   # [D, S]
    xl = np.ascontiguousarray(
        x[0, -1].reshape(DC, 128).T.astype(ml_dtypes.bfloat16))
    in_maps = []
    for i in range(NCORES):
        hs = slice(i * HP, (i + 1) * HP)
        in_maps.append({
            "xT": xT,
            "wq": np.ascontiguousarray(Wq[hs].astype(np.float16)),
            "wkv": np.ascontiguousarray(np.concatenate([
                Wk[hs].transpose(1, 0, 2).reshape(D, HP * DK),
                Wv[hs].transpose(1, 0, 2).reshape(D, HP * DK)],
                axis=1).astype(ml_dtypes.bfloat16)),
            "xl": xl,
            "bq": np.ascontiguousarray(bq[hs].reshape(HP, DK, 1)),
            "bkv": np.ascontiguousarray(np.concatenate(
                [bk[hs].reshape(-1), bv[hs].reshape(-1)])[None, :]),
            "kT": np.ascontiguousarray(
                k_cache[hs, :pos, :].transpose(0, 2, 1).astype(np.float16)),
            "v": np.ascontiguousarray(v_cache[hs, :pos, :].astype(np.float16)),
            "wo": np.ascontiguousarray(
                Wo[i * HP * DK:(i + 1) * HP * DK].astype(np.float16)),
        })

    if pos not in _CACHE:
        _CACHE[pos] = build(pos)
    nc = _CACHE[pos]

    res = run_bass_kernel_spmd(nc, in_maps, core_ids=list(range(NCORES)))
    LAST_EXEC_NS = res.exec_time_ns

    acc = np.zeros((S, D), np.float64)
    for r in res.results:
        acc += r["out"]
    out = (acc + bo.astype(np.float64)).astype(np.float32)
    return out[None]
